# revision 11
# baseline (speedup 1.0000x reference)
"""GroupPearson Trainium2 kernel, v7: PE diagonal-matmul segment reduction.

Sharding: host sorts elements by group id, pads each group to FG = T*128
elements, and ships two fp8(e4m3) streams (x=exp, y=pred) per core in a
"slab" layout with a baked ones-column: v[k, w, t, c] holds element
(128*t + k) of group (512*core + 128*w + c) for c < 128, and 1.0 for
c == 128.  Each core owns 512 groups = 4 psum-windows of 128 groups.

Device, per window w and slab t (stationary = x or y slab, groups as
columns; moving = 129-wide slab including the ones column):
  MM(psA, lhsT=x_t, rhs=[y_t|1]) -> diag += sum_k x*y ; col128 += sum_k x
  MM(psB, lhsT=x_t, rhs=x_t)     -> diag += sum_k x*x
  MM(psD, lhsT=y_t, rhs=[y_t|1]) -> diag += sum_k y*y ; col128 += sum_k y
accumulated over t in PSUM (fp32; only the first matmul into a bank may
carry start=True - start clears has_written for the whole bank).  DVE
extracts the diagonals with an identity-mask scalar_tensor_tensor
reduce and copies the ones-columns.  Output [P, NW*5] f32; host
finishes the correlation in float64.
"""

import numpy as np
import ml_dtypes

P = 128
C = 129                      # slab columns: 128 groups + ones column
G = 4096
N_CORES = 8
GPC = G // N_CORES           # 512 groups per core
NW = GPC // P                # 4 psum windows of 128 groups
NSUM = 5
NCK = 4                      # dma chunks per stream-window
USE_FP8 = True


def _enable_ldw_opt():
    # kept for reference; walrus ldw-opt is incompatible with the
    # pre-split InstLdweights that tile legalization emits.
    pass


def build_nc(T, n_devices=N_CORES):
    from concourse import mybir, tile, bacc
    from contextlib import ExitStack

    dt = mybir.dt
    OP = mybir.AluOpType
    ddt = dt.float8e4 if USE_FP8 else dt.bfloat16

    nc = bacc.Bacc("TRN2", target_bir_lowering=False, debug=False,
                   num_devices=n_devices)
    vxy_d = nc.dram_tensor("vxy", [P, NW, 2 * T * C], ddt,
                           kind="ExternalInput").ap()
    id_d = nc.dram_tensor("ident", [P, P], dt.bfloat16,
                          kind="ExternalInput").ap()
    o_d = nc.dram_tensor("o", [P, NW * NSUM], dt.float32,
                         kind="ExternalOutput").ap()

    with tile.TileContext(nc) as tc, ExitStack() as ctx:
        const_pool = ctx.enter_context(tc.tile_pool(name="const", bufs=1))
        out_pool = ctx.enter_context(tc.tile_pool(name="out", bufs=1))
        io_pool = ctx.enter_context(tc.tile_pool(name="io", bufs=NW))
        scr_pool = ctx.enter_context(tc.tile_pool(name="scr", bufs=2))
        ps_pool = ctx.enter_context(tc.psum_pool(name="ps", bufs=2))

        # issue all input DMAs first; x||y merged per window row for fat
        # DMA packets, dispatch alternating between the two HWDGE queues
        # (sync and scalar); everything stays resident (35KB/partition fp8)
        xyws = []
        for w in range(NW):
            xyw = io_pool.tile([P, 2 * T * C], ddt, tag="xyw")
            eng = nc.sync if w % 2 == 0 else nc.scalar
            eng.dma_start(out=xyw[:, :], in_=vxy_d[:, w, :])
            xyws.append(xyw)

        ident = const_pool.tile([P, P], dt.bfloat16)
        nc.sync.dma_start(out=ident[:, :], in_=id_d)

        outs = out_pool.tile([P, NW * NSUM], dt.float32)

        for w in range(NW):
            x3 = xyws[w][:, 0:T * C].rearrange("p (t c) -> p t c", c=C)
            y3 = xyws[w][:, T * C:2 * T * C].rearrange("p (t c) -> p t c",
                                                       c=C)

            # one PSUM bank: A | B | D  (each [P, C])
            ps = ps_pool.tile([P, 3 * C], dt.float32, tag="ps")
            psA = ps[:, 0:C]
            psB = ps[:, C:2 * C]
            psD = ps[:, 2 * C:3 * C]

            for t in range(T):
                xs = x3[:, t, 0:P]          # stationary: 128 group columns
                ys = y3[:, t, 0:P]
                xm = x3[:, t, :]            # moving: 129 cols (with ones)
                ym = y3[:, t, :]
                st = (t == 0)
                sp = (t == T - 1)
                nc.tensor.matmul(psA, lhsT=xs, rhs=ym, start=st, stop=sp,
                                 skip_group_check=True)
                nc.tensor.matmul(psB, lhsT=xs, rhs=xm, start=False, stop=sp,
                                 skip_group_check=True)
                nc.tensor.matmul(psD, lhsT=ys, rhs=ym, start=False, stop=sp,
                                 skip_group_check=True)

            ob = w * NSUM
            # diag extraction: accum_out = sum_n psum[m, n] * I[m, n]
            for j, psX in enumerate((psA, psB, psD)):
                scr = scr_pool.tile([P, P], dt.float32, tag=f"scr{j}")
                nc.vector.scalar_tensor_tensor(
                    scr[:, :], psX[:, 0:P], 1.0, ident[:, :], OP.mult,
                    OP.mult, accum_out=outs[:, ob + j:ob + j + 1])
            # ones-columns: psA col 128 = sx ; psD col 128 = sy
            nc.vector.tensor_copy(outs[:, ob + 3:ob + 4], psA[:, P:P + 1])
            nc.vector.tensor_copy(outs[:, ob + 4:ob + 5], psD[:, P:P + 1])

        nc.sync.dma_start(out=o_d[:, :], in_=outs[:, :])

    nc.compile()
    return nc


def host_layout(pred, exp, group):
    """Sorted+padded slab layout: per-core [P, NW, T*C] streams x and y."""
    from concourse import mybir
    ddt = mybir.dt.np(mybir.dt.float8e4) if USE_FP8 else ml_dtypes.bfloat16

    x = np.asarray(exp, dtype=np.float32)
    y = np.asarray(pred, dtype=np.float32)
    g = np.asarray(group).astype(np.int32)
    n = g.shape[0]

    sizes = np.bincount(g, minlength=G)
    T = max(int(np.ceil(int(sizes.max()) / P)), 1)
    FG = T * P
    order = np.argsort(g, kind="stable")
    gs = g[order].astype(np.int64)
    starts = np.zeros(G, dtype=np.int64)
    starts[1:] = np.cumsum(sizes)[:-1]
    pos = np.arange(n, dtype=np.int64) - starts[gs]
    dst = gs * FG + pos

    # merged [core, k, w, {x|y}, t, c] with baked ones column
    vxy = np.empty((N_CORES, P, NW, 2, T, C), dtype=ddt)
    for si, v in enumerate((x, y)):
        pad = np.zeros(G * FG, dtype=ddt)
        pad[dst] = v.astype(ddt)[order]
        # [core, w, c, t, k] -> [core, k, w, t, c]
        a = pad.reshape(N_CORES, NW, P, T, P)
        vxy[:, :, :, si, :, :P] = a.transpose(0, 4, 1, 3, 2)
    vxy[..., P] = ddt(1.0)
    return vxy.reshape(N_CORES, P, NW, 2 * T * C), sizes.astype(np.float64), T


def _finish_host(S):
    n, sx, sy, sxy, sxx, syy = S
    n_safe = np.where(n > 0, n, 1.0)
    mx = sx / n_safe
    my = sy / n_safe
    cov = sxy / n_safe - mx * my
    var_x = sxx / n_safe - mx * mx
    var_y = syy / n_safe - my * my
    denom = np.sqrt(np.maximum(var_x * var_y, 0.0))
    corr = np.where(denom > 0, cov / np.where(denom > 0, denom, 1.0), 0.0)
    corr_pearson = np.sum(corr * n) / np.sum(n)
    return np.float32(-corr_pearson)


_NC_CACHE = {}


def _get_nc(T):
    if T not in _NC_CACHE:
        _NC_CACHE[T] = build_nc(T)
    return _NC_CACHE[T]


def kernel(pred, exp, group, num_groups, _trace=False):
    from concourse.bass_utils import run_bass_kernel_spmd

    pred = np.asarray(pred)
    exp = np.asarray(exp)
    group = np.asarray(group)

    vxy, sizes, T = host_layout(pred, exp, group)
    nc = _get_nc(T)

    ident = np.eye(P, dtype=ml_dtypes.bfloat16)
    in_maps = [{"vxy": vxy[i], "ident": ident}
               for i in range(N_CORES)]

    res = run_bass_kernel_spmd(nc, in_maps, list(range(N_CORES)),
                               trace=_trace)

    S = np.zeros((6, G), dtype=np.float64)
    S[0] = sizes
    for i in range(N_CORES):
        o = res.results[i]["o"].astype(np.float64)  # [P, NW*5]
        o = o.reshape(P, NW, NSUM)                  # partition=group-in-window
        # group = 512*i + 128*w + m  ; stats order: sxy, sxx, syy, sx, sy
        blk = o.transpose(2, 1, 0).reshape(NSUM, GPC)  # [5, w*128+m]
        S[3, GPC * i:GPC * (i + 1)] = blk[0]
        S[4, GPC * i:GPC * (i + 1)] = blk[1]
        S[5, GPC * i:GPC * (i + 1)] = blk[2]
        S[1, GPC * i:GPC * (i + 1)] = blk[3]
        S[2, GPC * i:GPC * (i + 1)] = blk[4]
    out = _finish_host(S)
    if _trace:
        return out, res
    return out


# revision 14
# speedup vs baseline: 1.1527x; 1.1527x over previous
"""GroupPearson Trainium2 kernel, v10: PE 2x2-Gram diagonal segment reduction.

Sharding: host sorts elements by group id, pads each group to FG = T*128
elements, and ships one fp8(e4m3) tensor per core in a "Gram slab"
layout: each of 8 windows covers 64 groups; slab (w, t) is a [128, 129]
block whose columns are [x of 64 groups | y of 64 groups | ones], rows
are 128 consecutive elements.  Each core owns 512 groups.

Device, per window w and slab t — ONE matmul with
  lhsT = slab[:, 0:128]  ([x64 | y64]),  rhs = slab[:, 0:129]:
  psum[m,      m     ] += sum_k x*x   (sxx)
  psum[m,      64 + m] += sum_k x*y   (sxy)
  psum[64 + m, 64 + m] += sum_k y*y   (syy)
  psum[m,      128   ] += sum_k x     (sx)
  psum[64 + m, 128   ] += sum_k y     (sy)
accumulated over t in fp32 PSUM (only the first matmul into a bank may
carry start=True - start clears has_written for the whole bank).  DVE
extracts the three diagonals with an identity-mask
scalar_tensor_tensor reduce and copies the ones-column halves.
Output [P, 8*3] f32; host finishes the correlation in float64.
"""

import numpy as np
import ml_dtypes

P = 128
H = 64                       # groups per window
C = 129                      # slab columns: x(64) | y(64) | ones
G = 4096
N_CORES = 8
GPC = G // N_CORES           # 512 groups per core
NW = GPC // H                # 8 windows of 64 groups
NO = 3                       # output cols per window
USE_FP8 = True


def build_nc(T, n_devices=N_CORES):
    from concourse import mybir, tile, bacc
    from contextlib import ExitStack

    dt = mybir.dt
    OP = mybir.AluOpType
    ddt = dt.float8e4 if USE_FP8 else dt.bfloat16

    nc = bacc.Bacc("TRN2", target_bir_lowering=False, debug=False,
                   num_devices=n_devices)
    v_d = nc.dram_tensor("v", [P, NW, T * C], ddt,
                         kind="ExternalInput").ap()
    id_d = nc.dram_tensor("ident", [P, H], dt.bfloat16,
                          kind="ExternalInput").ap()
    o_d = nc.dram_tensor("o", [P, NW * NO], dt.float32,
                         kind="ExternalOutput").ap()

    with tile.TileContext(nc) as tc, ExitStack() as ctx:
        const_pool = ctx.enter_context(tc.tile_pool(name="const", bufs=1))
        out_pool = ctx.enter_context(tc.tile_pool(name="out", bufs=1))
        io_pool = ctx.enter_context(tc.tile_pool(name="io", bufs=1))
        scr_pool = ctx.enter_context(tc.tile_pool(name="scr", bufs=2))
        ps_pool = ctx.enter_context(tc.psum_pool(name="ps", bufs=4))

        # input DMAs first: window 0 and 1 individually (fast fill), then
        # window pairs with fat 2*T*C rows; alternate the two HWDGE queues
        ws = []
        t0 = io_pool.tile([P, T * C], ddt, tag="w0")
        nc.sync.dma_start(out=t0[:, :], in_=v_d[:, 0, :])
        ws.append(t0[:, :])
        t1 = io_pool.tile([P, T * C], ddt, tag="w1")
        nc.scalar.dma_start(out=t1[:, :], in_=v_d[:, 1, :])
        ws.append(t1[:, :])
        for k in range(1, NW // 2):
            pair = io_pool.tile([P, 2 * T * C], ddt, tag=f"pair{k}")
            eng = nc.sync if k % 2 == 1 else nc.scalar
            eng.dma_start(out=pair[:, :], in_=v_d[:, 2 * k:2 * k + 2, :])
            ws.append(pair[:, 0:T * C])
            ws.append(pair[:, T * C:2 * T * C])

        ident = const_pool.tile([P, H], dt.bfloat16)
        nc.sync.dma_start(out=ident[:, :], in_=id_d)

        outs = out_pool.tile([P, NW * NO], dt.float32)

        for w in range(NW):
            s3 = ws[w].rearrange("p (t c) -> p t c", c=C)

            ps = ps_pool.tile([P, C], dt.float32, tag="ps")
            for t in range(T):
                nc.tensor.matmul(ps[:, :], lhsT=s3[:, t, 0:P],
                                 rhs=s3[:, t, :], start=(t == 0),
                                 stop=(t == T - 1), skip_group_check=True)

            ob = w * NO
            # diagonals: sxx (rows 0:64), syy (rows 64:128), sxy (0:64)
            scr = scr_pool.tile([P, H], dt.float32, tag="scr")
            nc.vector.scalar_tensor_tensor(
                scr[0:H, :], ps[0:H, 0:H], 1.0, ident[0:H, :], OP.mult,
                OP.mult, accum_out=outs[0:H, ob:ob + 1])
            nc.vector.scalar_tensor_tensor(
                scr[H:P, :], ps[H:P, H:2 * H], 1.0, ident[H:P, :], OP.mult,
                OP.mult, accum_out=outs[H:P, ob:ob + 1])
            scr2 = scr_pool.tile([P, H], dt.float32, tag="scr2")
            nc.vector.scalar_tensor_tensor(
                scr2[0:H, :], ps[0:H, H:2 * H], 1.0, ident[0:H, :], OP.mult,
                OP.mult, accum_out=outs[0:H, ob + 1:ob + 2])
            # ones column: sx (rows 0:64), sy (rows 64:128)
            nc.vector.tensor_copy(outs[:, ob + 2:ob + 3], ps[:, P:P + 1])

        nc.sync.dma_start(out=o_d[:, :], in_=outs[:, :])

    nc.compile()
    return nc


def host_layout(pred, exp, group):
    """Sorted+padded Gram-slab layout: per-core [P, NW, T*C] fp8."""
    from concourse import mybir
    ddt = mybir.dt.np(mybir.dt.float8e4) if USE_FP8 else ml_dtypes.bfloat16

    x = np.asarray(exp, dtype=np.float32)
    y = np.asarray(pred, dtype=np.float32)
    g = np.asarray(group).astype(np.int32)
    n = g.shape[0]

    sizes = np.bincount(g, minlength=G)
    T = max(int(np.ceil(int(sizes.max()) / P)), 1)
    FG = T * P
    order = np.argsort(g, kind="stable")
    gs = g[order].astype(np.int64)
    starts = np.zeros(G, dtype=np.int64)
    starts[1:] = np.cumsum(sizes)[:-1]
    pos = np.arange(n, dtype=np.int64) - starts[gs]
    dst = gs * FG + pos

    v = np.empty((N_CORES, P, NW, T, C), dtype=ddt)
    for si, vv in enumerate((x, y)):
        pad = np.zeros(G * FG, dtype=ddt)
        pad[dst] = vv.astype(ddt)[order]
        # [core, w, c, t, k] -> [core, k, w, t, c]
        a = pad.reshape(N_CORES, NW, H, T, P)
        v[:, :, :, :, si * H:(si + 1) * H] = a.transpose(0, 4, 1, 3, 2)
    v[..., 2 * H] = ddt(1.0)
    return v.reshape(N_CORES, P, NW, T * C), sizes.astype(np.float64), T


def _finish_host(S):
    n, sx, sy, sxy, sxx, syy = S
    n_safe = np.where(n > 0, n, 1.0)
    mx = sx / n_safe
    my = sy / n_safe
    cov = sxy / n_safe - mx * my
    var_x = sxx / n_safe - mx * mx
    var_y = syy / n_safe - my * my
    denom = np.sqrt(np.maximum(var_x * var_y, 0.0))
    corr = np.where(denom > 0, cov / np.where(denom > 0, denom, 1.0), 0.0)
    corr_pearson = np.sum(corr * n) / np.sum(n)
    return np.float32(-corr_pearson)


_NC_CACHE = {}


def _get_nc(T):
    if T not in _NC_CACHE:
        _NC_CACHE[T] = build_nc(T)
    return _NC_CACHE[T]


def kernel(pred, exp, group, num_groups, _trace=False):
    from concourse.bass_utils import run_bass_kernel_spmd

    pred = np.asarray(pred)
    exp = np.asarray(exp)
    group = np.asarray(group)

    v, sizes, T = host_layout(pred, exp, group)
    nc = _get_nc(T)

    idh = np.eye(H, dtype=ml_dtypes.bfloat16)
    ident = np.concatenate([idh, idh], axis=0)          # [128, 64]
    in_maps = [{"v": v[i], "ident": ident} for i in range(N_CORES)]

    res = run_bass_kernel_spmd(nc, in_maps, list(range(N_CORES)),
                               trace=_trace)

    S = np.zeros((6, G), dtype=np.float64)
    S[0] = sizes
    for i in range(N_CORES):
        o = res.results[i]["o"].astype(np.float64)      # [P, NW*3]
        o = o.reshape(P, NW, NO)
        sl = slice(GPC * i, GPC * (i + 1))
        # group = 512*i + 64*w + m
        S[4, sl] = o[0:H, :, 0].T.reshape(GPC)          # sxx
        S[5, sl] = o[H:P, :, 0].T.reshape(GPC)          # syy
        S[3, sl] = o[0:H, :, 1].T.reshape(GPC)          # sxy
        S[1, sl] = o[0:H, :, 2].T.reshape(GPC)          # sx
        S[2, sl] = o[H:P, :, 2].T.reshape(GPC)          # sy
    out = _finish_host(S)
    if _trace:
        return out, res
    return out


# revision 16
# speedup vs baseline: 1.3089x; 1.1355x over previous
"""GroupPearson Trainium2 kernel, v10: PE 2x2-Gram diagonal segment reduction.

Sharding: host sorts elements by group id, pads each group to FG = T*128
elements, and ships one fp8(e4m3) tensor per core in a "Gram slab"
layout: each of 8 windows covers 64 groups; slab (w, t) is a [128, 129]
block whose columns are [x of 64 groups | y of 64 groups | ones], rows
are 128 consecutive elements.  Each core owns 512 groups.

Device, per window w and slab t — ONE matmul with
  lhsT = slab[:, 0:128]  ([x64 | y64]),  rhs = slab[:, 0:129]:
  psum[m,      m     ] += sum_k x*x   (sxx)
  psum[m,      64 + m] += sum_k x*y   (sxy)
  psum[64 + m, 64 + m] += sum_k y*y   (syy)
  psum[m,      128   ] += sum_k x     (sx)
  psum[64 + m, 128   ] += sum_k y     (sy)
accumulated over t in fp32 PSUM (only the first matmul into a bank may
carry start=True - start clears has_written for the whole bank).  DVE
extracts the three diagonals with an identity-mask
scalar_tensor_tensor reduce and copies the ones-column halves.
Output [P, 8*3] f32; host finishes the correlation in float64.
"""

import numpy as np
import ml_dtypes

P = 128
H = 64                       # groups per window
C = 129                      # slab columns: x(64) | y(64) | ones
G = 4096
N_CORES = 8
GPC = G // N_CORES           # 512 groups per core
NW = GPC // H                # 8 windows of 64 groups
NO = 3                       # output cols per window
USE_FP8 = True


def build_nc(T, n_devices=N_CORES):
    from concourse import mybir, tile, bacc
    from contextlib import ExitStack

    dt = mybir.dt
    OP = mybir.AluOpType
    ddt = dt.float8e4 if USE_FP8 else dt.bfloat16

    nc = bacc.Bacc("TRN2", target_bir_lowering=False, debug=False,
                   num_devices=n_devices)
    v_d = nc.dram_tensor("v", [P, NW, T * C], ddt,
                         kind="ExternalInput").ap()
    id_d = nc.dram_tensor("ident", [P, H], dt.bfloat16,
                          kind="ExternalInput").ap()
    o_d = nc.dram_tensor("o", [P, NW * NO], dt.float32,
                         kind="ExternalOutput").ap()

    with tile.TileContext(nc) as tc, ExitStack() as ctx:
        const_pool = ctx.enter_context(tc.tile_pool(name="const", bufs=1))
        out_pool = ctx.enter_context(tc.tile_pool(name="out", bufs=1))
        io_pool = ctx.enter_context(tc.tile_pool(name="io", bufs=1))
        scr_pool = ctx.enter_context(tc.tile_pool(name="scr", bufs=2))
        ps_pool = ctx.enter_context(tc.psum_pool(name="ps", bufs=4))

        # input DMAs first: window 0 and 1 individually (fast fill), then
        # window pairs with fat 2*T*C rows; alternate the two HWDGE queues
        ws = []
        t0 = io_pool.tile([P, T * C], ddt, tag="w0")
        nc.sync.dma_start(out=t0[:, :], in_=v_d[:, 0, :])
        ws.append(t0[:, :])
        t1 = io_pool.tile([P, T * C], ddt, tag="w1")
        nc.scalar.dma_start(out=t1[:, :], in_=v_d[:, 1, :])
        ws.append(t1[:, :])
        for k in range(1, NW // 2):
            pair = io_pool.tile([P, 2 * T * C], ddt, tag=f"pair{k}")
            eng = nc.sync if k % 2 == 1 else nc.scalar
            eng.dma_start(out=pair[:, :], in_=v_d[:, 2 * k:2 * k + 2, :])
            ws.append(pair[:, 0:T * C])
            ws.append(pair[:, T * C:2 * T * C])

        ident = const_pool.tile([P, H], dt.bfloat16)
        nc.sync.dma_start(out=ident[:, :], in_=id_d)

        outs = out_pool.tile([P, NW * NO], dt.float32)

        def extract(ps, w):
            ob = w * NO
            # diagonals: sxx (rows 0:64), syy (rows 64:128), sxy (0:64)
            scr = scr_pool.tile([P, H], dt.float32, tag="scr")
            nc.vector.scalar_tensor_tensor(
                scr[0:H, :], ps[0:H, 0:H], 1.0, ident[0:H, :], OP.mult,
                OP.mult, accum_out=outs[0:H, ob:ob + 1])
            nc.vector.scalar_tensor_tensor(
                scr[H:P, :], ps[H:P, H:2 * H], 1.0, ident[H:P, :], OP.mult,
                OP.mult, accum_out=outs[H:P, ob:ob + 1])
            scr2 = scr_pool.tile([P, H], dt.float32, tag="scr2")
            nc.vector.scalar_tensor_tensor(
                scr2[0:H, :], ps[0:H, H:2 * H], 1.0, ident[0:H, :], OP.mult,
                OP.mult, accum_out=outs[0:H, ob + 1:ob + 2])
            # ones column: sx (rows 0:64), sy (rows 64:128)
            nc.vector.tensor_copy(outs[:, ob + 2:ob + 3], ps[:, P:P + 1])

        # window pairs with interleaved slabs: MM(wa,t), MM(wb,t), ... so
        # each matmul hides the other stream's LDWEIGHTS
        for k in range(NW // 2):
            wa, wb = 2 * k, 2 * k + 1
            sa = ws[wa].rearrange("p (t c) -> p t c", c=C)
            sb = ws[wb].rearrange("p (t c) -> p t c", c=C)
            pa = ps_pool.tile([P, C], dt.float32, tag="psa")
            pb = ps_pool.tile([P, C], dt.float32, tag="psb")
            for t in range(T):
                nc.tensor.matmul(pa[:, :], lhsT=sa[:, t, 0:P],
                                 rhs=sa[:, t, :], start=(t == 0),
                                 stop=(t == T - 1), skip_group_check=True)
                nc.tensor.matmul(pb[:, :], lhsT=sb[:, t, 0:P],
                                 rhs=sb[:, t, :], start=(t == 0),
                                 stop=(t == T - 1), skip_group_check=True)
            extract(pa, wa)
            extract(pb, wb)

        nc.sync.dma_start(out=o_d[:, :], in_=outs[:, :])

    nc.compile()
    return nc


def host_layout(pred, exp, group):
    """Sorted+padded Gram-slab layout: per-core [P, NW, T*C] fp8."""
    from concourse import mybir
    ddt = mybir.dt.np(mybir.dt.float8e4) if USE_FP8 else ml_dtypes.bfloat16

    x = np.asarray(exp, dtype=np.float32)
    y = np.asarray(pred, dtype=np.float32)
    g = np.asarray(group).astype(np.int32)
    n = g.shape[0]

    sizes = np.bincount(g, minlength=G)
    T = max(int(np.ceil(int(sizes.max()) / P)), 1)
    FG = T * P
    order = np.argsort(g, kind="stable")
    gs = g[order].astype(np.int64)
    starts = np.zeros(G, dtype=np.int64)
    starts[1:] = np.cumsum(sizes)[:-1]
    pos = np.arange(n, dtype=np.int64) - starts[gs]
    dst = gs * FG + pos

    v = np.empty((N_CORES, P, NW, T, C), dtype=ddt)
    for si, vv in enumerate((x, y)):
        pad = np.zeros(G * FG, dtype=ddt)
        pad[dst] = vv.astype(ddt)[order]
        # [core, w, c, t, k] -> [core, k, w, t, c]
        a = pad.reshape(N_CORES, NW, H, T, P)
        v[:, :, :, :, si * H:(si + 1) * H] = a.transpose(0, 4, 1, 3, 2)
    v[..., 2 * H] = ddt(1.0)
    return v.reshape(N_CORES, P, NW, T * C), sizes.astype(np.float64), T


def _finish_host(S):
    n, sx, sy, sxy, sxx, syy = S
    n_safe = np.where(n > 0, n, 1.0)
    mx = sx / n_safe
    my = sy / n_safe
    cov = sxy / n_safe - mx * my
    var_x = sxx / n_safe - mx * mx
    var_y = syy / n_safe - my * my
    denom = np.sqrt(np.maximum(var_x * var_y, 0.0))
    corr = np.where(denom > 0, cov / np.where(denom > 0, denom, 1.0), 0.0)
    corr_pearson = np.sum(corr * n) / np.sum(n)
    return np.float32(-corr_pearson)


_NC_CACHE = {}


def _get_nc(T):
    if T not in _NC_CACHE:
        _NC_CACHE[T] = build_nc(T)
    return _NC_CACHE[T]


def kernel(pred, exp, group, num_groups, _trace=False):
    from concourse.bass_utils import run_bass_kernel_spmd

    pred = np.asarray(pred)
    exp = np.asarray(exp)
    group = np.asarray(group)

    v, sizes, T = host_layout(pred, exp, group)
    nc = _get_nc(T)

    idh = np.eye(H, dtype=ml_dtypes.bfloat16)
    ident = np.concatenate([idh, idh], axis=0)          # [128, 64]
    in_maps = [{"v": v[i], "ident": ident} for i in range(N_CORES)]

    res = run_bass_kernel_spmd(nc, in_maps, list(range(N_CORES)),
                               trace=_trace)

    S = np.zeros((6, G), dtype=np.float64)
    S[0] = sizes
    for i in range(N_CORES):
        o = res.results[i]["o"].astype(np.float64)      # [P, NW*3]
        o = o.reshape(P, NW, NO)
        sl = slice(GPC * i, GPC * (i + 1))
        # group = 512*i + 64*w + m
        S[4, sl] = o[0:H, :, 0].T.reshape(GPC)          # sxx
        S[5, sl] = o[H:P, :, 0].T.reshape(GPC)          # syy
        S[3, sl] = o[0:H, :, 1].T.reshape(GPC)          # sxy
        S[1, sl] = o[0:H, :, 2].T.reshape(GPC)          # sx
        S[2, sl] = o[H:P, :, 2].T.reshape(GPC)          # sy
    out = _finish_host(S)
    if _trace:
        return out, res
    return out
